# revision 25
# baseline (speedup 1.0000x reference)
"""Trainium2 Bass kernel for nn_ConvSepKanCell (per-pixel dynamic KAN layer).

Layout: channels on SBUF partitions, tokens on the free dim.  v3: w is
converted to bf16 and repacked on the host so each 512-token chunk is one
contiguous [128, 26*T] DMA (26KB per partition line); all constant matrices
ship as one packed DMA; the chunk loop is fully unrolled with double-
buffered bases/w so chunk k+1's basis evaluation overlaps chunk k's coef
phase.  Every PE matmul runs 1-cycle-per-row: bf16 operands, or float32r
for the x broadcast (basis positions keep fp32 storage).  The cubic
B-spline bases are translates of the cardinal spline M3(u), evaluated with
ScalarE activations (Abs/Relu/Square) and DVE multiplies in bf16.
Partition-broadcasts (x -> 176 basis rows -> 2816 coef rows) and the m/i
contractions run on the TensorEngine against constant 0/1 selection
matrices, accumulating in PSUM (fp32).  Data-parallel over 8 cores: each
core owns 4096 contiguous tokens of one batch image.
"""

import sys

sys.path.insert(0, "/opt/trn_rl_repo")

import ml_dtypes
import numpy as np

import concourse.bass as bass
import concourse.mybir as mybir
from concourse.bass_utils import run_bass_kernel_spmd
from concourse.tile import TileContext

F32 = mybir.dt.float32
F32R = mybir.dt.float32r
BF16 = mybir.dt.bfloat16
AF = mybir.ActivationFunctionType
NPBF16 = ml_dtypes.bfloat16

# Pin every Tile HW-DMA to bookkeeping lane 0: fewer distinct DMA sems means
# stage waits aggregate one wait instead of eight (this walrus build has
# tight per-instruction sync-wait limits).  Hardware throughput is
# unaffected: lanes are completion bookkeeping, all SP DMAs share one HWDGE
# ring that fans out across the 16 SDMA engines.
import concourse.tile_sem_assignment as _tsa

_orig_assign_tick = _tsa.TileClockTick._assign_tick


def _patched_assign_tick(self, inst):
    if isinstance(inst, _tsa.DMAInst) and inst.engine != mybir.EngineType.Pool:
        self.next_hw_dma_idx = 0
    return _orig_assign_tick(self, inst)


_tsa.TileClockTick._assign_tick = _patched_assign_tick


IN_C, OUT_C, M = 16, 16, 11
NCOEF = IN_C * OUT_C * M          # 2816 = 22 * 128
SIZE = NCOEF + 2 * IN_C * OUT_C   # 3328 = 26 * 128
NB = 176                          # basis rows = 16 i * 11 m
T = 512                           # tokens per chunk
NCHUNK = 8
TOK = T * NCHUNK                  # 4096 tokens per core
NCORES = 8
NG = NCOEF // 128                 # 22 coef channel groups
NW = SIZE // 128                  # 26 w channel groups

ALPHA = float((1.0 / 6.0) ** (1.0 / 3.0))
BETA = float((2.0 / 3.0) ** (1.0 / 3.0))


NB0 = 121                         # basis rows i*11+m for i<=10
NB1 = NB - NB0                    # 55 rows for i>=11


def _consts():
    c = np.arange(NCOEF)
    io = c // M
    rowmap = (io // OUT_C) * M + c % M      # basis row for each coef channel
    r = np.arange(NB)
    bias_v = (5.0 - (r % M)).astype(np.float32)  # v = |4x + 7 - m - 2|

    Mg0 = np.zeros((NG, NB0, 128), np.float32)
    Mg1 = np.zeros((NG, NB1, 128), np.float32)
    Rg = np.zeros((NG, 128, 128), np.float32)
    for g in range(NG):
        rm = rowmap[128 * g : 128 * (g + 1)]
        p = np.arange(128)
        lo = rm < NB0
        Mg0[g, rm[lo], p[lo]] = 1.0
        Mg1[g, rm[~lo] - NB0, p[~lo]] = 1.0
        col = io[128 * g : 128 * (g + 1)] - (128 if g >= M else 0)
        Rg[g, p, col] = 1.0

    S2 = np.zeros((16, 256), np.float32)    # silu row -> io rows
    ioo = np.arange(256)
    S2[ioo // OUT_C, ioo] = 1.0
    F = np.zeros((128, 16), np.float32)     # io rows -> o
    F[np.arange(128), np.arange(128) % 16] = 1.0
    return dict(bias_v=bias_v, Mg0=Mg0, Mg1=Mg1, Rg=Rg, S2=S2, F=F)


_CACHE = {}


def _build():
    if "nc" in _CACHE:
        return _CACHE["nc"]
    C = _consts()
    nc = bass.Bass()
    xs = nc.dram_tensor("x", [16, TOK], F32, kind="ExternalInput")
    ws = nc.dram_tensor("w", [128, NW, TOK], BF16, kind="ExternalInput")
    ys = nc.dram_tensor("y", [16, TOK], F32, kind="ExternalOutput")

    # ---- pack all bf16 constant matrices into one [128, ncol] blob ----
    blocks = []          # (key, col0, ncols, nrows)
    col = 0
    def add(key, mat):
        nonlocal col
        nr, ncl = mat.shape
        blocks.append((key, col, ncl, nr))
        col += ncl
    for g in range(NG):
        if C["Mg0"][g].any():
            add(f"Mg0_{g}", C["Mg0"][g])
        if C["Mg1"][g].any():
            add(f"Mg1_{g}", C["Mg1"][g])
        add(f"Rg_{g}", C["Rg"][g])
    add("S2a", C["S2"][:, :128])
    add("S2b", C["S2"][:, 128:])
    add("F", C["F"])
    blob = np.zeros((128, col), np.float32)
    for key, c0, ncl, nr in blocks:
        src = {"S2a": C["S2"][:, :128], "S2b": C["S2"][:, 128:], "F": C["F"]}
        if key in src:
            blob[:nr, c0:c0 + ncl] = src[key]
        else:
            kind, g = key.rsplit("_", 1)
            blob[:nr, c0:c0 + ncl] = C[kind][int(g)]
    d_blob = nc.inline_tensor(blob.astype(NPBF16), name="c_blob")
    biasf = np.stack([
        np.pad(C["bias_v"][:NB0], (0, 128 - NB0)),
        np.pad(C["bias_v"][NB0:], (0, 128 - NB1)),
        np.full(128, 2.0 * ALPHA, np.float32),
        np.full(128, BETA, np.float32),
    ], axis=1)
    d_bias = nc.inline_tensor(np.ascontiguousarray(biasf), name="c_bias")

    sb_blob = nc.alloc_sbuf_tensor("s_blob", [128, col], BF16)
    sb_bias = nc.alloc_sbuf_tensor("s_bias", [128, 4], F32)
    sb_x = nc.alloc_sbuf_tensor("s_x", [16, TOK], F32)
    with nc.semaphore() as csem:
        nc.sync.dma_start(out=sb_blob.ap(), in_=d_blob[:]).then_inc(csem, 16)
        nc.sync.dma_start(out=sb_bias.ap(), in_=d_bias[:]).then_inc(csem, 16)
        nc.sync.dma_start(out=sb_x.ap(), in_=xs[:]).then_inc(csem, 16)
        nc.sync.wait_ge(csem, 16 * 3)
    nc.all_engine_barrier()

    cmap = {key: sb_blob.ap()[:nr, c0:c0 + ncl]
            for key, c0, ncl, nr in blocks}
    bias_v0 = sb_bias.ap()[:NB0, 0:1]
    bias_v1 = sb_bias.ap()[:NB1, 1:2]
    b_s1 = sb_bias.ap()[:, 2:3]
    b_s2 = sb_bias.ap()[:, 3:4]
    S2a = cmap["S2a"]; S2b = cmap["S2b"]; Fm = cmap["F"]
    Mg0 = [cmap.get(f"Mg0_{g}") for g in range(NG)]
    Mg1 = [cmap.get(f"Mg1_{g}") for g in range(NG)]
    Rg = [cmap[f"Rg_{g}"] for g in range(NG)]

    with TileContext(nc) as tc:
        with (
            tc.tile_pool(name="wq", bufs=2) as wqp,
            tc.tile_pool(name="small", bufs=2) as sp,
            tc.tile_pool(name="bas", bufs=2) as bp,
            tc.tile_pool(name="prod", bufs=22) as pp,
            tc.tile_pool(name="tt", bufs=2) as tp,
            tc.tile_pool(name="pxs", bufs=2, space="PSUM") as pxs,
            tc.tile_pool(name="pbe", bufs=3, space="PSUM") as pbe,
            tc.tile_pool(name="pspl", bufs=2, space="PSUM") as pspl,
            tc.tile_pool(name="py", bufs=1, space="PSUM") as pyp,
        ):
            quads = [(0, 4), (4, 4), (8, 4), (12, 4), (16, 4), (20, 6)]
            with tc.For_i(0, TOK, T, staggered_reset=True) as off:
                tok = bass.ds(off, T)

                # ---- bases on 176 rows: x -> 11 copies per i via broadcast
                # DMA straight from HBM (no PE involvement, so chunk k+1's
                # ScalarE basis chain overlaps chunk k's coef phase) ----
                xe0 = bp.tile([NB0, T], F32, tag="xe0")
                nc.sync.dma_start(
                    out=xe0[:],
                    in_=xs[0:11, tok].rearrange("i t -> i () t")
                        .broadcast_to([11, M, T]))
                xe1 = bp.tile([NB1, T], F32, tag="xe1")
                nc.sync.dma_start(
                    out=xe1[:],
                    in_=xs[11:16, tok].rearrange("i t -> i () t")
                        .broadcast_to([5, M, T]))

                bases = []
                for xe, bias, nrow, sfx in ((xe0, bias_v0, NB0, "0"),
                                            (xe1, bias_v1, NB1, "1")):
                    v = bp.tile([nrow, T], F32, tag=f"v{sfx}")
                    nc.scalar.activation(v[:], xe[:], AF.Abs,
                                         bias=bias, scale=4.0)
                    s1 = bp.tile([nrow, T], BF16, tag=f"s1{sfx}")
                    nc.scalar.activation(s1[:], v[:], AF.Relu,
                                         bias=b_s1[:nrow], scale=-ALPHA)
                    s2 = bp.tile([nrow, T], BF16, tag=f"s2{sfx}")
                    nc.scalar.activation(s2[:], v[:], AF.Relu,
                                         bias=b_s2[:nrow], scale=-BETA)
                    q1 = bp.tile([nrow, T], BF16, tag=f"q1{sfx}")
                    nc.scalar.activation(q1[:], s1[:], AF.Square)
                    q2 = bp.tile([nrow, T], BF16, tag=f"q2{sfx}")
                    nc.scalar.activation(q2[:], s2[:], AF.Square)
                    c1 = bp.tile([nrow, T], BF16, tag=f"c1{sfx}")
                    nc.vector.tensor_mul(c1[:], q1[:], s1[:])
                    c2 = bp.tile([nrow, T], BF16, tag=f"c2{sfx}")
                    nc.vector.tensor_mul(c2[:], q2[:], s2[:])
                    b = bp.tile([nrow, T], BF16, tag=f"b{sfx}")
                    nc.vector.tensor_sub(b[:], c1[:], c2[:])
                    bases.append(b)
                b0, b1 = bases

                # silu early on ScalarE so the late se matmuls never stall PE
                sx = sp.tile([16, T], BF16, tag="sx")
                nc.scalar.activation(sx[:], sb_x.ap()[:, tok], AF.Silu)
                tc.stage_boundary()

                # ---- coef groups ----
                spl0 = pspl.tile([128, T], F32, tag="pspl")
                spl1 = pspl.tile([128, T], F32, tag="pspl")
                uwrw = None
                for q0, qn in quads:
                    wt = wqp.tile([128, qn, T], BF16,
                                  tag=("wq6" if qn == 6 else "wq"),
                                  bufs=(2 if qn == 6 else 10), name="wt")
                    nc.sync.dma_start(out=wt[:], in_=ws[:, q0:q0 + qn, tok])
                    if q0 == 20:
                        uwrw = wt
                    # same-engine touch absorbs the DMA wait so later DVE
                    # ops reading wt carry only their compute waits (walrus
                    # sync-wait count limit)
                    wtv = bp.tile([128, 1], BF16, tag="wtv", bufs=12,
                                  name="wtv")
                    nc.vector.tensor_copy(wtv[:], wt[:, 0, :1])
                    for j in range(min(qn, NG - q0)):
                        g = q0 + j
                        be = pbe.tile([128, T], F32, tag="pbe")
                        first = True
                        if Mg0[g] is not None:
                            nc.tensor.matmul(
                                be[:], Mg0[g], b0[:],
                                start=True, stop=(Mg1[g] is None),
                                skip_group_check=True)
                            first = False
                        if Mg1[g] is not None:
                            nc.tensor.matmul(
                                be[:], Mg1[g], b1[:],
                                start=first, stop=True,
                                skip_group_check=True)
                        pg = pp.tile([128, T], BF16, tag="prod")
                        nc.vector.tensor_mul(pg[:], wt[:, j, :], be[:])
                        spl = spl0 if g < M else spl1
                        nc.tensor.matmul(
                            spl[:], Rg[g], pg[:],
                            start=(g % M == 0), stop=(g % M == M - 1),
                            skip_group_check=True)

                # ---- residual path (late: PE never waits on ScalarE) ----
                se0 = pxs.tile([128, T], F32, tag="pxs")
                nc.tensor.matmul(se0[:], S2a, sx[:], start=True, stop=True,
                                 skip_group_check=True)
                se1 = pxs.tile([128, T], F32, tag="pxs")
                nc.tensor.matmul(se1[:], S2b, sx[:], start=True, stop=True,
                                 skip_group_check=True)

                # ---- combine ----
                t1_0 = tp.tile([128, T], BF16, tag="t1_0")
                nc.vector.tensor_mul(t1_0[:], uwrw[:, 2, :], spl0[:])
                t1_1 = tp.tile([128, T], BF16, tag="t1_1")
                nc.vector.tensor_mul(t1_1[:], uwrw[:, 3, :], spl1[:])
                t2_0 = tp.tile([128, T], BF16, tag="t2_0")
                nc.vector.tensor_mul(t2_0[:], uwrw[:, 4, :], se0[:])
                t2_1 = tp.tile([128, T], BF16, tag="t2_1")
                nc.vector.tensor_mul(t2_1[:], uwrw[:, 5, :], se1[:])

                yp = pyp.tile([16, T], F32, tag="py")
                nc.tensor.matmul(yp[:], Fm, t1_0[:], start=True, stop=False,
                                 skip_group_check=True)
                nc.tensor.matmul(yp[:], Fm, t1_1[:], start=False, stop=False,
                                 skip_group_check=True)
                nc.tensor.matmul(yp[:], Fm, t2_0[:], start=False, stop=False,
                                 skip_group_check=True)
                nc.tensor.matmul(yp[:], Fm, t2_1[:], start=False, stop=True,
                                 skip_group_check=True)
                y_sb = sp.tile([16, T], F32, tag="y")
                nc.vector.tensor_copy(y_sb[:], yp[:])
                tc.stage_boundary()
                nc.sync.dma_start(out=ys[:, tok], in_=y_sb[:])
                tc.stage_boundary()

    _CACHE["nc"] = nc
    return nc


LAST = {}


def kernel(x, w):
    x = np.asarray(x, np.float32)
    w = np.asarray(w, np.float32)
    nc = _build()
    x2 = x.reshape(2, 16, 16384)
    w2 = w.reshape(2, SIZE, 16384)
    in_maps = []
    for k in range(NCORES):
        b, s = k // 4, (k % 4) * TOK
        wc = w2[b, :, s : s + TOK]
        wpack = np.ascontiguousarray(
            wc.reshape(NW, 128, TOK).transpose(1, 0, 2)
        ).astype(NPBF16)
        in_maps.append({
            "x": np.ascontiguousarray(x2[b, :, s : s + TOK]),
            "w": wpack,
        })
    res = run_bass_kernel_spmd(nc, in_maps, list(range(NCORES)),
                               trace=LAST.get("trace", False))
    LAST["results"] = res
    out = np.empty((2, 16, 16384), np.float32)
    for k in range(NCORES):
        b, s = k // 4, (k % 4) * TOK
        out[b][:, s : s + TOK] = res.results[k]["y"]
    return out.reshape(2, 16, 128, 128)


# revision 28
# speedup vs baseline: 1.1218x; 1.1218x over previous
"""Trainium2 Bass kernel for nn_ConvSepKanCell (per-pixel dynamic KAN layer).

Layout: channels on SBUF partitions, tokens on the free dim.  v3: w is
converted to bf16 and repacked on the host so each 512-token chunk is one
contiguous [128, 26*T] DMA (26KB per partition line); all constant matrices
ship as one packed DMA; the chunk loop is fully unrolled with double-
buffered bases/w so chunk k+1's basis evaluation overlaps chunk k's coef
phase.  Every PE matmul runs 1-cycle-per-row: bf16 operands, or float32r
for the x broadcast (basis positions keep fp32 storage).  The cubic
B-spline bases are translates of the cardinal spline M3(u), evaluated with
ScalarE activations (Abs/Relu/Square) and DVE multiplies in bf16.
Partition-broadcasts (x -> 176 basis rows -> 2816 coef rows) and the m/i
contractions run on the TensorEngine against constant 0/1 selection
matrices, accumulating in PSUM (fp32).  Data-parallel over 8 cores: each
core owns 4096 contiguous tokens of one batch image.
"""

import sys

sys.path.insert(0, "/opt/trn_rl_repo")

import ml_dtypes
import numpy as np

import concourse.bass as bass
import concourse.mybir as mybir
from concourse.bass_utils import run_bass_kernel_spmd
from concourse.tile import TileContext

F32 = mybir.dt.float32
F32R = mybir.dt.float32r
BF16 = mybir.dt.bfloat16
AF = mybir.ActivationFunctionType
NPBF16 = ml_dtypes.bfloat16

# Pin every Tile HW-DMA to bookkeeping lane 0: fewer distinct DMA sems means
# stage waits aggregate one wait instead of eight (this walrus build has
# tight per-instruction sync-wait limits).  Hardware throughput is
# unaffected: lanes are completion bookkeeping, all SP DMAs share one HWDGE
# ring that fans out across the 16 SDMA engines.
import concourse.tile_sem_assignment as _tsa

_orig_assign_tick = _tsa.TileClockTick._assign_tick


def _patched_assign_tick(self, inst):
    if isinstance(inst, _tsa.DMAInst) and inst.engine != mybir.EngineType.Pool:
        self.next_hw_dma_idx = 0
    return _orig_assign_tick(self, inst)


_tsa.TileClockTick._assign_tick = _patched_assign_tick


IN_C, OUT_C, M = 16, 16, 11
NCOEF = IN_C * OUT_C * M          # 2816 = 22 * 128
SIZE = NCOEF + 2 * IN_C * OUT_C   # 3328 = 26 * 128
NB = 176                          # basis rows = 16 i * 11 m
T = 512                           # tokens per chunk
NCHUNK = 8
TOK = T * NCHUNK                  # 4096 tokens per core
NCORES = 8
NG = NCOEF // 128                 # 22 coef channel groups
NW = SIZE // 128                  # 26 w channel groups

ALPHA = float((1.0 / 6.0) ** (1.0 / 3.0))
BETA = float((2.0 / 3.0) ** (1.0 / 3.0))


NB0 = 121                         # basis rows i*11+m for i<=10
NB1 = NB - NB0                    # 55 rows for i>=11


def _consts():
    c = np.arange(NCOEF)
    io = c // M
    rowmap = (io // OUT_C) * M + c % M      # basis row for each coef channel
    r = np.arange(NB)
    bias_v = (5.0 - (r % M)).astype(np.float32)  # v = |4x + 7 - m - 2|

    Mg0 = np.zeros((NG, NB0, 128), np.float32)
    Mg1 = np.zeros((NG, NB1, 128), np.float32)
    Rg = np.zeros((NG, 128, 128), np.float32)
    for g in range(NG):
        rm = rowmap[128 * g : 128 * (g + 1)]
        p = np.arange(128)
        lo = rm < NB0
        Mg0[g, rm[lo], p[lo]] = 1.0
        Mg1[g, rm[~lo] - NB0, p[~lo]] = 1.0
        col = io[128 * g : 128 * (g + 1)] - (128 if g >= M else 0)
        Rg[g, p, col] = 1.0

    S2 = np.zeros((16, 256), np.float32)    # silu row -> io rows
    ioo = np.arange(256)
    S2[ioo // OUT_C, ioo] = 1.0
    F = np.zeros((128, 16), np.float32)     # io rows -> o
    F[np.arange(128), np.arange(128) % 16] = 1.0
    return dict(bias_v=bias_v, Mg0=Mg0, Mg1=Mg1, Rg=Rg, S2=S2, F=F)


_CACHE = {}


def _build():
    if "nc" in _CACHE:
        return _CACHE["nc"]
    C = _consts()
    nc = bass.Bass()
    xs = nc.dram_tensor("x", [16, TOK], F32, kind="ExternalInput")
    ws = nc.dram_tensor("w", [128, NW, TOK], BF16, kind="ExternalInput")
    ys = nc.dram_tensor("y", [16, TOK], F32, kind="ExternalOutput")

    # ---- pack all bf16 constant matrices into one [128, ncol] blob ----
    blocks = []          # (key, col0, ncols, nrows)
    col = 0
    def add(key, mat):
        nonlocal col
        nr, ncl = mat.shape
        blocks.append((key, col, ncl, nr))
        col += ncl
    for g in range(NG):
        if C["Mg0"][g].any():
            add(f"Mg0_{g}", C["Mg0"][g])
        if C["Mg1"][g].any():
            add(f"Mg1_{g}", C["Mg1"][g])
        add(f"Rg_{g}", C["Rg"][g])
    add("S2a", C["S2"][:, :128])
    add("S2b", C["S2"][:, 128:])
    add("F", C["F"])
    blob = np.zeros((128, col), np.float32)
    for key, c0, ncl, nr in blocks:
        src = {"S2a": C["S2"][:, :128], "S2b": C["S2"][:, 128:], "F": C["F"]}
        if key in src:
            blob[:nr, c0:c0 + ncl] = src[key]
        else:
            kind, g = key.rsplit("_", 1)
            blob[:nr, c0:c0 + ncl] = C[kind][int(g)]
    d_blob = nc.inline_tensor(blob.astype(NPBF16), name="c_blob")
    biasf = np.stack([
        np.pad(C["bias_v"][:NB0], (0, 128 - NB0)),
        np.pad(C["bias_v"][NB0:], (0, 128 - NB1)),
        np.full(128, 2.0 * ALPHA, np.float32),
        np.full(128, BETA, np.float32),
    ], axis=1)
    d_bias = nc.inline_tensor(np.ascontiguousarray(biasf), name="c_bias")

    sb_blob = nc.alloc_sbuf_tensor("s_blob", [128, col], BF16)
    sb_bias = nc.alloc_sbuf_tensor("s_bias", [128, 4], F32)
    sb_x = nc.alloc_sbuf_tensor("s_x", [16, TOK], F32)
    with nc.semaphore() as csem:
        nc.sync.dma_start(out=sb_blob.ap(), in_=d_blob[:]).then_inc(csem, 16)
        nc.sync.dma_start(out=sb_bias.ap(), in_=d_bias[:]).then_inc(csem, 16)
        nc.sync.dma_start(out=sb_x.ap(), in_=xs[:]).then_inc(csem, 16)
        nc.sync.wait_ge(csem, 16 * 3)
    nc.all_engine_barrier()

    cmap = {key: sb_blob.ap()[:nr, c0:c0 + ncl]
            for key, c0, ncl, nr in blocks}
    bias_v0 = sb_bias.ap()[:NB0, 0:1]
    bias_v1 = sb_bias.ap()[:NB1, 1:2]
    b_s1 = sb_bias.ap()[:, 2:3]
    b_s2 = sb_bias.ap()[:, 3:4]
    S2a = cmap["S2a"]; S2b = cmap["S2b"]; Fm = cmap["F"]
    Mg0 = [cmap.get(f"Mg0_{g}") for g in range(NG)]
    Mg1 = [cmap.get(f"Mg1_{g}") for g in range(NG)]
    Rg = [cmap[f"Rg_{g}"] for g in range(NG)]

    with TileContext(nc) as tc:
        with (
            tc.tile_pool(name="wq", bufs=2) as wqp,
            tc.tile_pool(name="small", bufs=2) as sp,
            tc.tile_pool(name="bas", bufs=2) as bp,
            tc.tile_pool(name="prod", bufs=22) as pp,
            tc.tile_pool(name="tt", bufs=2) as tp,
            tc.tile_pool(name="pxs", bufs=2, space="PSUM") as pxs,
            tc.tile_pool(name="pbe", bufs=3, space="PSUM") as pbe,
            tc.tile_pool(name="pspl", bufs=2, space="PSUM") as pspl,
            tc.tile_pool(name="py", bufs=1, space="PSUM") as pyp,
        ):
            quads = [(0, 4), (4, 4), (8, 4), (12, 4), (16, 4), (20, 6)]
            y_acc = nc.alloc_sbuf_tensor("y_acc", [16, TOK], F32)
            with tc.For_i(0, TOK, T, staggered_reset=True) as off:
                tok = bass.ds(off, T)

                # ---- bases on 176 rows: x -> 11 copies per i via broadcast
                # DMA straight from HBM (no PE involvement, so chunk k+1's
                # ScalarE basis chain overlaps chunk k's coef phase) ----
                xe0 = bp.tile([NB0, T], F32, tag="xe0")
                nc.sync.dma_start(
                    out=xe0[:],
                    in_=xs[0:11, tok].rearrange("i t -> i () t")
                        .broadcast_to([11, M, T]))
                xe1 = bp.tile([NB1, T], F32, tag="xe1")
                nc.sync.dma_start(
                    out=xe1[:],
                    in_=xs[11:16, tok].rearrange("i t -> i () t")
                        .broadcast_to([5, M, T]))

                bases = []
                for xe, bias, nrow, sfx in ((xe0, bias_v0, NB0, "0"),
                                            (xe1, bias_v1, NB1, "1")):
                    v = bp.tile([nrow, T], F32, tag=f"v{sfx}")
                    nc.scalar.activation(v[:], xe[:], AF.Abs,
                                         bias=bias, scale=4.0)
                    s1 = bp.tile([nrow, T], BF16, tag=f"s1{sfx}")
                    nc.scalar.activation(s1[:], v[:], AF.Relu,
                                         bias=b_s1[:nrow], scale=-ALPHA)
                    s2 = bp.tile([nrow, T], BF16, tag=f"s2{sfx}")
                    nc.scalar.activation(s2[:], v[:], AF.Relu,
                                         bias=b_s2[:nrow], scale=-BETA)
                    q1 = bp.tile([nrow, T], BF16, tag=f"q1{sfx}")
                    nc.scalar.activation(q1[:], s1[:], AF.Square)
                    q2 = bp.tile([nrow, T], BF16, tag=f"q2{sfx}")
                    nc.scalar.activation(q2[:], s2[:], AF.Square)
                    c1 = bp.tile([nrow, T], BF16, tag=f"c1{sfx}")
                    nc.vector.tensor_mul(c1[:], q1[:], s1[:])
                    c2 = bp.tile([nrow, T], BF16, tag=f"c2{sfx}")
                    nc.vector.tensor_mul(c2[:], q2[:], s2[:])
                    b = bp.tile([nrow, T], BF16, tag=f"b{sfx}")
                    nc.vector.tensor_sub(b[:], c1[:], c2[:])
                    bases.append(b)
                b0, b1 = bases

                # silu early on ScalarE so the late se matmuls never stall PE
                sx = sp.tile([16, T], BF16, tag="sx")
                nc.scalar.activation(sx[:], sb_x.ap()[:, tok], AF.Silu)

                # ---- coef groups ----
                spl0 = pspl.tile([128, T], F32, tag="pspl")
                spl1 = pspl.tile([128, T], F32, tag="pspl")
                uwrw = None
                for q0, qn in quads:
                    wt = wqp.tile([128, qn, T], BF16,
                                  tag=("wq6" if qn == 6 else "wq"),
                                  bufs=(2 if qn == 6 else 10), name="wt")
                    nc.sync.dma_start(out=wt[:], in_=ws[:, q0:q0 + qn, tok])
                    if q0 == 20:
                        uwrw = wt
                    # same-engine touch absorbs the DMA wait so later DVE
                    # ops reading wt carry only their compute waits (walrus
                    # sync-wait count limit)
                    wtv = bp.tile([128, 1], BF16, tag="wtv", bufs=12,
                                  name="wtv")
                    nc.vector.tensor_copy(wtv[:], wt[:, 0, :1])
                    for j in range(min(qn, NG - q0)):
                        g = q0 + j
                        be = pbe.tile([128, T], F32, tag="pbe")
                        first = True
                        if Mg0[g] is not None:
                            nc.tensor.matmul(
                                be[:], Mg0[g], b0[:],
                                start=True, stop=(Mg1[g] is None),
                                skip_group_check=True)
                            first = False
                        if Mg1[g] is not None:
                            nc.tensor.matmul(
                                be[:], Mg1[g], b1[:],
                                start=first, stop=True,
                                skip_group_check=True)
                        pg = pp.tile([128, T], BF16, tag="prod")
                        nc.vector.tensor_mul(pg[:], wt[:, j, :], be[:])
                        spl = spl0 if g < M else spl1
                        nc.tensor.matmul(
                            spl[:], Rg[g], pg[:],
                            start=(g % M == 0), stop=(g % M == M - 1),
                            skip_group_check=True)

                # ---- residual path (late: PE never waits on ScalarE) ----
                se0 = pxs.tile([128, T], F32, tag="pxs")
                nc.tensor.matmul(se0[:], S2a, sx[:], start=True, stop=True,
                                 skip_group_check=True)
                se1 = pxs.tile([128, T], F32, tag="pxs")
                nc.tensor.matmul(se1[:], S2b, sx[:], start=True, stop=True,
                                 skip_group_check=True)

                # ---- combine ----
                t1_0 = tp.tile([128, T], BF16, tag="t1_0")
                nc.vector.tensor_mul(t1_0[:], uwrw[:, 2, :], spl0[:])
                t1_1 = tp.tile([128, T], BF16, tag="t1_1")
                nc.vector.tensor_mul(t1_1[:], uwrw[:, 3, :], spl1[:])
                t2_0 = tp.tile([128, T], BF16, tag="t2_0")
                nc.vector.tensor_mul(t2_0[:], uwrw[:, 4, :], se0[:])
                t2_1 = tp.tile([128, T], BF16, tag="t2_1")
                nc.vector.tensor_mul(t2_1[:], uwrw[:, 5, :], se1[:])

                yp = pyp.tile([16, T], F32, tag="py")
                nc.tensor.matmul(yp[:], Fm, t1_0[:], start=True, stop=False,
                                 skip_group_check=True)
                nc.tensor.matmul(yp[:], Fm, t1_1[:], start=False, stop=False,
                                 skip_group_check=True)
                nc.tensor.matmul(yp[:], Fm, t2_0[:], start=False, stop=False,
                                 skip_group_check=True)
                nc.tensor.matmul(yp[:], Fm, t2_1[:], start=False, stop=True,
                                 skip_group_check=True)
                nc.vector.tensor_copy(y_acc.ap()[:, tok], yp[:])
                tc.stage_boundary()
                tc.stage_boundary()
                tc.stage_boundary()
            # single post-loop writeback: one wait (the in-loop copy inst)
            nc.sync.dma_start(out=ys[:], in_=y_acc.ap())

    _CACHE["nc"] = nc
    return nc


LAST = {}


def kernel(x, w):
    x = np.asarray(x, np.float32)
    w = np.asarray(w, np.float32)
    nc = _build()
    x2 = x.reshape(2, 16, 16384)
    w2 = w.reshape(2, SIZE, 16384)
    in_maps = []
    for k in range(NCORES):
        b, s = k // 4, (k % 4) * TOK
        wc = w2[b, :, s : s + TOK]
        wpack = np.ascontiguousarray(
            wc.reshape(NW, 128, TOK).transpose(1, 0, 2)
        ).astype(NPBF16)
        in_maps.append({
            "x": np.ascontiguousarray(x2[b, :, s : s + TOK]),
            "w": wpack,
        })
    res = run_bass_kernel_spmd(nc, in_maps, list(range(NCORES)),
                               trace=LAST.get("trace", False))
    LAST["results"] = res
    out = np.empty((2, 16, 16384), np.float32)
    for k in range(NCORES):
        b, s = k // 4, (k % 4) * TOK
        out[b][:, s : s + TOK] = res.results[k]["y"]
    return out.reshape(2, 16, 128, 128)
